# revision 1
# baseline (speedup 1.0000x reference)
"""Trainium2 Bass kernel for nn_Attn_spa (dense transformer attention with
pre-computed bias logits), SPMD over 8 NeuronCores.

Sharding: core c handles batch b = c//2 and head-half hh = c%2 (8 of 16 heads).
Per-core device program (all layouts keep seq as the free dim):
  preT = silu(Wpre.T @ xT + bpre)            [C,N]   (transposed pre)
  qT/kT = (Wq_h.T/8) @ xT                    [512,N]
  v    = xT.T @ Wv_h                         [N,512] (natural)
  L    = preT.T @ preT                       [N,N]   (bias logits, symmetric)
  per head h: sT = kT_h.T @ qT_h             [m,n]  (scoreT tile-by-tile)
              u  = exp(pi/32*L + sT)         (unnormalized attn, transposed)
              uo[d,n] += v_h[m,d].T @ u ; den[n] += 1.T @ u
              outT_h = uo * (1/den)          (broadcast via ones-matmul)
  y_partial = outT.T @ Wproj_h               [N,C]
Host: y[b] = y_partial(core 2b) + y_partial(core 2b+1) + x[b] + bproj.

Precision: matmul inputs are float32r (full-rate fp32 on the PE; producers
round on write) except the attention-weight path (exp output u and V) which
is bf16; the reciprocal/pi broadcasts stay exact fp32. Measured end-to-end
relative error vs the fp32 reference: ~8e-4.

Constraints baked into the structure (found the hard way):
- float32r matmuls must target PSUM partition base 0 (walrus s3d3 check), so
  every head computes in rows 0..63 and odd heads DMA-shift into outt; heads
  run odd-first (1,3,5,7,0,2,4,6) so no shift-DMA sits on the proj critical
  path.
- DVE/ACT ops need identical partition bases on all operands (lane-tied).
- A `start=True` matmul clears has_written for its whole PSUM bank, so
  interleaved accumulation groups must live in different banks (V carries a
  ones column so attn@V also emits the softmax denominator, row 64).
- Early phases consume DMA chunk arrivals with up to 8 open [128,512] PSUM
  groups (arrival-major, ci pairs) so the PE tracks DMA line rate at start.
"""

import sys

sys.path.insert(0, "/opt/trn_rl_repo")

import numpy as np

B, N, C = 4, 1024, 1024
H, DH = 16, 64
NCORES = 8
CH = C // 2  # features per core in the head-sharded dim (8 heads * 64)

USE_F32R = True
GP_MULS = 0  # of 8 per-head exp*EL muls, how many go to GPSIMD
UT_BF16 = True  # attn weights + V in bf16 (faster DVE mul, ~10x precision cost)
BC_ON_ACT = True
UT_BUFS = 6
UR_BUFS = 3
U_BUFS = 3
D_BUFS = 1
SG_BUFS = 4
Y_BUFS = 3
NORM_AT = 99  # mid-head norm emission disabled: scheduler-neutral

_cached = {}


def _build_nc(repeat=1):
    import concourse.bass as bass
    import concourse.mybir as mybir
    import concourse.tile as tile
    from concourse import bacc

    f32 = mybir.dt.float32
    f32r = mybir.dt.float32r
    AF = mybir.ActivationFunctionType
    ALU = mybir.AluOpType

    mmdt = f32r if USE_F32R else f32
    utdt = mybir.dt.bfloat16 if UT_BF16 else mmdt

    nc = bacc.Bacc("TRN2", target_bir_lowering=False, debug=False)

    xt_d = nc.dram_tensor("xt", [C, N], mmdt, kind="ExternalInput")
    wpre_d = nc.dram_tensor("wpre", [C, C], mmdt, kind="ExternalInput")
    wq_d = nc.dram_tensor("wq", [C, CH], mmdt, kind="ExternalInput")
    wk_d = nc.dram_tensor("wk", [C, CH], mmdt, kind="ExternalInput")
    wv_d = nc.dram_tensor("wv", [C, CH], mmdt, kind="ExternalInput")
    wproj_d = nc.dram_tensor("wproj", [CH, C], mmdt, kind="ExternalInput")
    bpre_d = nc.dram_tensor("bpre", [C], f32, kind="ExternalInput")
    pi_d = nc.dram_tensor("pi", [1, 1], f32, kind="ExternalInput")
    y_d = nc.dram_tensor("y", [N, C], f32, kind="ExternalOutput")

    with tile.TileContext(nc) as tc:
      from contextlib import ExitStack

      for _rep in range(repeat):
        with ExitStack() as ctx:
            work0 = ctx.enter_context(tc.tile_pool(name="work0", bufs=1))
            pearly_cm = tc.tile_pool(name="pse", bufs=1, space="PSUM")
            pearly = pearly_cm.__enter__()

            def chunks(name, n, shape, side="right", dt=None):
                # SBUF pools are per-side LIFO stacks: allocate long-lived
                # tensors on the right, phase-temporaries on the left in
                # reverse-free order.
                tiles, frees = [], []
                for i in range(n):
                    t, f = tc.tile(shape, dt or mmdt, name=f"{name}{i}", side=side)
                    tiles.append(t)
                    frees.append(f)
                return tiles, (lambda fl=frees: [f() for f in reversed(fl)])

            # ---- constants (bottom of the right stack, freed last) ----
            ones32_sb, free_ones32 = tc.tile([128, 128], f32, name="ones32", side="right")
            nc.vector.memset(ones32_sb[:], 1.0)
            pi_sb, free_pi = tc.tile([1, 1], f32, name="pisb", side="right")
            bpre_sb, free_bpre = tc.tile([128, 8], f32, name="bpresb", side="right")

            # ---- load inputs ----
            # left-stack alloc order = reverse free order:
            # pre (freed last) < xt < wq < wk < wv < wpre (freed first)
            pre_sb, free_pre = chunks("pre", 8, [128, N], side="left")
            xt_sb, free_xt = chunks("xt", 8, [128, N], side="left")
            wq_sb, free_wq = chunks("wq", 8, [128, CH], side="left")
            wk_sb, free_wk = chunks("wk", 8, [128, CH], side="left")
            wv_sb, free_wv = chunks("wv", 8, [128, CH], side="left")
            wpre_sb, free_wpre = chunks("wpre", 8, [128, C], side="left")
            nc.sync.dma_start(wv_sb[0][:, 0:256], wv_d[0:128, 0:256])
            nc.sync.dma_start(xt_sb[0][:, 0:512], xt_d[0:128, 0:512])
            nc.sync.dma_start(wv_sb[0][:, 256:512], wv_d[0:128, 256:512])
            nc.sync.dma_start(xt_sb[0][:, 512:1024], xt_d[0:128, 512:1024])
            for i in range(1, 8):
                nc.sync.dma_start(wv_sb[i][:], wv_d[128 * i : 128 * (i + 1), :])
                nc.sync.dma_start(xt_sb[i][:], xt_d[128 * i : 128 * (i + 1), :])
            for i in range(8):
                nc.sync.dma_start(wpre_sb[i][:], wpre_d[128 * i : 128 * (i + 1), :])
            for i in range(8):
                nc.sync.dma_start(wq_sb[i][:], wq_d[128 * i : 128 * (i + 1), :])
                nc.sync.dma_start(wk_sb[i][:], wk_d[128 * i : 128 * (i + 1), :])
            # tiny loads via SWDGE (gpsimd): HWDGE descriptor-gen is 625ns
            # per DMA, serialized — on nc.sync these would delay the
            # critical first wv/xt chunks
            nc.gpsimd.dma_start(pi_sb[0:1, 0:1], pi_d[:, :])
            nc.gpsimd.dma_start(bpre_sb[:, :], bpre_d.rearrange("(c p) -> p c", p=128))

            # ---- phase V first (smallest weights): arrival-major over ci,
            # 8 open nv accumulation groups so the PE consumes each DMA chunk
            # as it lands ----
            v_sb, free_v = chunks("v", 8, [128, 8 * 65], dt=utdt)
            v_ps = [pearly.tile([128, 512], f32, tag="a", bufs=8, name=f"vps{nv}")
                    for nv in range(8)]
            for cip in range(2):
                for nv in range(8):
                    for ci in range(4 * cip, 4 * cip + 4):
                        if ci == 0:
                            # ci=0 split on wv halves so the PE starts on
                            # partially-arrived first-chunk DMAs; second
                            # half start=False (a second start=True would
                            # clear the whole bank's has_written bits)
                            nc.tensor.matmul(
                                v_ps[nv][:, 0:256],
                                xt_sb[0][:, 128 * nv : 128 * (nv + 1)],
                                wv_sb[0][:, 0:256],
                                start=True, stop=False,
                            )
                            nc.tensor.matmul(
                                v_ps[nv][:, 256:512],
                                xt_sb[0][:, 128 * nv : 128 * (nv + 1)],
                                wv_sb[0][:, 256:512],
                                start=False, stop=False,
                            )
                        else:
                            nc.tensor.matmul(
                                v_ps[nv][:],
                                xt_sb[ci][:, 128 * nv : 128 * (nv + 1)],
                                wv_sb[ci][:],
                                start=False, stop=(ci == 7),
                            )
            for nv in range(8):
                # v stored as [128, 8*65]: per head 64 value cols + a ones col
                # (makes the attn@V matmul emit the softmax denom as row 64);
                # copies alternate ACT/DVE so the 8 pearly slots free faster
                v3 = v_sb[nv].rearrange("p (h d) -> p h d", d=65)
                cpeng = nc.scalar.copy if nv % 2 == 0 else nc.vector.tensor_copy
                cpeng(v3[:, :, 0:64], v_ps[nv][:].rearrange("p (h d) -> p h d", d=64))
                if UT_BF16:
                    nc.vector.memset(v_sb[nv][:, 64::65], 1.0)
                else:
                    nc.scalar.copy(v_sb[nv][:, 64::65], ones32_sb[:, 0:8])

            # ---- phase A: preT = silu(Wpre.T @ xT + bpre), two waves of 8
            # open (co,half) groups, arrival-major over ci ----
            for wave in range(2):
                a_ps = {}
                for g in range(8):
                    co, half = 4 * wave + g // 2, g % 2
                    a_ps[g] = pearly.tile(
                        [128, 512], f32, tag="a", bufs=8, name=f"aps{wave}_{g}"
                    )
                for cip in range(2):
                    for g in range(8):
                        co, half = 4 * wave + g // 2, g % 2
                        for ci in range(4 * cip, 4 * cip + 4):
                            nc.tensor.matmul(
                                a_ps[g][:],
                                wpre_sb[ci][:, 128 * co : 128 * (co + 1)],
                                xt_sb[ci][:, 512 * half : 512 * (half + 1)],
                                start=(ci == 0), stop=(ci == 7),
                            )
                for g in range(8):
                    co, half = 4 * wave + g // 2, g % 2
                    hs = slice(512 * half, 512 * (half + 1))
                    # silu(z) = z*sigmoid(z), z = psum + bpre (sim lacks Silu)
                    sg = work0.tile([128, 512], f32, tag="sg", bufs=SG_BUFS)
                    nc.scalar.activation(
                        sg[:], a_ps[g][:], AF.Sigmoid, bias=bpre_sb[:, co : co + 1]
                    )
                    nc.vector.scalar_tensor_tensor(
                        pre_sb[co][:, hs], a_ps[g][:], bpre_sb[:, co : co + 1],
                        sg[:], ALU.add, ALU.mult,
                    )
            free_wpre()

            # ---- phase A2: qT, kT (transposed), 8 open groups each ----
            qt_sb, free_qt = chunks("qt", 4, [128, N])  # right side
            kt_sb, free_kt = chunks("kt", 4, [128, N])
            for dst, w_sb in ((qt_sb, wq_sb), (kt_sb, wk_sb)):
                q_ps = {}
                for g in range(8):
                    q_ps[g] = pearly.tile(
                        [128, 512], f32, tag="a", bufs=8,
                        name=f"qps{dst[0].tensor.name}_{g}",
                    )
                for cip in range(2):
                    for g in range(8):
                        cq, half = g // 2, g % 2
                        for ci in range(4 * cip, 4 * cip + 4):
                            nc.tensor.matmul(
                                q_ps[g][:],
                                w_sb[ci][:, 128 * cq : 128 * (cq + 1)],
                                xt_sb[ci][:, 512 * half : 512 * (half + 1)],
                                start=(ci == 0), stop=(ci == 7),
                            )
                for g in range(8):
                    cq, half = g // 2, g % 2
                    nc.scalar.copy(
                        dst[cq][:, 512 * half : 512 * (half + 1)], q_ps[g][:]
                    )
            pearly_cm.__exit__(None, None, None)
            ppool = ctx.enter_context(tc.tile_pool(name="ps", bufs=1, space="PSUM"))
            # pi broadcast to all 128 partitions via PE, then * 1/sqrt(C)
            # (deferred here: first needed by phase B's EL)
            pi_ps = ppool.tile([128, 1], f32, tag="d", bufs=D_BUFS)
            nc.tensor.matmul(
                pi_ps[:, 0:1], ones32_sb[0:1, 0:128], pi_sb[0:1, 0:1],
                start=True, stop=True,
            )
            pi32_sb, free_pi32 = tc.tile([128, 1], f32, name="pi32", side="right")
            nc.scalar.activation(pi32_sb[:], pi_ps[:], AF.Copy, scale=1.0 / 32.0)
            free_wv()
            free_wk()
            free_wq()
            free_xt()

            # ---- phase B: L = preT.T @ preT, with head 1's attention
            # pipeline interleaved per m (its exp stream hides under B's
            # PE-bound window; score needs only qt/kt, mul needs el[m] just
            # produced; one spare s-slot allows exactly one head here) ----
            el_sb, free_l = chunks("el", 8, [128, N], dt=(mybir.dt.bfloat16 if UT_BF16 else f32))
            u1_ps = [
                ppool.tile([128, 512], f32, tag="u", bufs=U_BUFS, name=f"u1ps{t}")
                for t in range(2)
            ]
            ut3_list = []
            for m in range(8):
                ps = ppool.tile([128, 1024], f32, tag="s", bufs=2)
                for half in range(2):
                    for c in range(8):
                        nc.tensor.matmul(
                            ps[:, 512 * half : 512 * (half + 1)],
                            pre_sb[c][:, 128 * m : 128 * (m + 1)],
                            pre_sb[c][:, 512 * half : 512 * (half + 1)],
                            start=(c == 0), stop=(c == 7),
                        )
                # EL = exp(pi/32 * L) so u = exp(score)*EL later
                nc.scalar.activation(
                    el_sb[m][:], ps[:], AF.Exp, scale=pi32_sb[:, 0:1]
                )
                ur1 = work0.tile(
                    [128, 1024], mybir.dt.bfloat16 if UT_BF16 else f32,
                    tag="ur1", bufs=UR_BUFS,
                )
                for half in range(2):
                    s1 = ppool.tile(
                        [128, 512], f32, tag="d", bufs=D_BUFS, name=f"s1ps{m}{half}"
                    )
                    nc.tensor.matmul(
                        s1[:],
                        kt_sb[0][64:128, 128 * m : 128 * (m + 1)],
                        qt_sb[0][64:128, 512 * half : 512 * (half + 1)],
                        start=True, stop=True,
                    )
                    nc.scalar.activation(
                        ur1[:, 512 * half : 512 * (half + 1)], s1[:], AF.Exp
                    )
                ut1 = work0.tile([128, 1024], utdt, tag="ut1", bufs=UT_BUFS)
                nc.vector.tensor_mul(ut1[:], ur1[:], el_sb[m][:])
                for t in range(2):
                    nc.tensor.matmul(
                        u1_ps[t][0:65, :],
                        v_sb[m][:, 65 * 1 : 65 * 1 + 65],
                        ut1[:, 512 * t : 512 * (t + 1)],
                        start=(m == 0), stop=(m == 7),
                    )
                # head 3's score/exp/mul hide here too; scores ride the u
                # tag's spare slot (d is saturated by head 1's stream) and
                # the attn weights are stashed for phase D's U matmuls
                ur3 = work0.tile(
                    [128, 1024], mybir.dt.bfloat16 if UT_BF16 else f32,
                    tag="ur1", bufs=UR_BUFS, name=f"ur3_{m}",
                )
                for half in range(2):
                    s3 = ppool.tile(
                        [128, 512], f32, tag="u", bufs=U_BUFS, name=f"s3ps{m}{half}"
                    )
                    nc.tensor.matmul(
                        s3[:],
                        kt_sb[1][64:128, 128 * m : 128 * (m + 1)],
                        qt_sb[1][64:128, 512 * half : 512 * (half + 1)],
                        start=True, stop=True,
                    )
                    nc.scalar.activation(
                        ur3[:, 512 * half : 512 * (half + 1)], s3[:], AF.Exp
                    )
                ut3 = work0.tile(
                    [128, 1024], utdt, tag="ut3", bufs=8, name=f"ut3_{m}"
                )
                nc.vector.tensor_mul(ut3[:], ur3[:], el_sb[m][:])
                ut3_list.append(ut3)
            free_pre()

            wproj_sb, free_wproj = chunks("wproj", 4, [128, C])
            for i in range(4):
                nc.sync.dma_start(wproj_sb[i][:], wproj_d[128 * i : 128 * (i + 1), :])
            outt_sb, free_outt = chunks("outt", 4, [128, N])

            # ---- phase D: per-head attention ----
            with tc.tile_pool(name="work", bufs=1) as work:
                def emit_norm(u_ps, hc, hb):
                    for t in range(2):
                        recip = work.tile([128, 512], f32, tag="rc", bufs=2)
                        nc.vector.reciprocal(recip[64:65, :], u_ps[t][64:65, :])
                        # broadcast 1/den over 64 partitions (fp32 PE matmul)
                        d_ps = ppool.tile([128, 512], f32, tag="d", bufs=D_BUFS)
                        nc.tensor.matmul(
                            d_ps[0:64, :],
                            ones32_sb[64:65, 0:64],
                            recip[64:65, :],
                            start=True, stop=True,
                        )
                        bc = work.tile([128, 512], f32, tag="bc", bufs=2)
                        bceng = (
                            nc.scalar.copy
                            if (BC_ON_ACT if isinstance(BC_ON_ACT, bool) else t == 0)
                            else nc.vector.tensor_copy
                        )
                        bceng(bc[0:64, :], d_ps[0:64, :])
                        if hb == 0:
                            nc.vector.tensor_mul(
                                outt_sb[hc][0:64, 512 * t : 512 * (t + 1)],
                                u_ps[t][0:64, :],
                                bc[0:64, :],
                            )
                        else:
                            shift = work.tile([128, 512], mmdt, tag="sh", bufs=2)
                            nc.vector.tensor_mul(
                                shift[0:64, :], u_ps[t][0:64, :], bc[0:64, :]
                            )
                            nc.sync.dma_start(
                                outt_sb[hc][64:128, 512 * t : 512 * (t + 1)],
                                shift[0:64, :],
                            )

                pending = (u1_ps, 0, 64)  # head 1, computed during phase B
                for h in (3, 5, 7, 0, 2, 4, 6):
                    # f32r matmuls require dst partition base 0, and DVE ops
                    # need matching partition bases — so every head computes
                    # in rows 0..63; odd heads DMA-shift into outt rows 64..127.
                    hb = (h % 2) * 64
                    hc = h // 2
                    u_ps = [
                        ppool.tile([128, 512], f32, tag="u", bufs=U_BUFS, name=f"ups{h}_{t}")
                        for t in range(2)
                    ]
                    for m in range(8):
                        if h == 3:
                            ut = ut3_list[m]
                        else:
                            s_ps = ppool.tile([128, 1024], f32, tag="s", bufs=2)
                            for half in range(2):
                                nc.tensor.matmul(
                                    s_ps[:, 512 * half : 512 * (half + 1)],
                                    kt_sb[hc][hb : hb + 64, 128 * m : 128 * (m + 1)],
                                    qt_sb[hc][hb : hb + 64, 512 * half : 512 * (half + 1)],
                                    start=True, stop=True,
                                )
                            ur = work.tile([128, 1024], mybir.dt.bfloat16 if UT_BF16 else f32, tag="ur", bufs=UR_BUFS)
                            nc.scalar.activation(ur[:], s_ps[:], AF.Exp)
                            ut = work.tile([128, 1024], utdt, tag="ut", bufs=UT_BUFS)
                            eng = nc.gpsimd if m >= 8 - GP_MULS else nc.vector
                            eng.tensor_mul(ut[:], ur[:], el_sb[m][:])
                        for t in range(2):
                            nc.tensor.matmul(
                                u_ps[t][0:65, :],
                                v_sb[m][:, 65 * h : 65 * h + 65],
                                ut[:, 512 * t : 512 * (t + 1)],
                                start=(m == 0), stop=(m == 7),
                            )
                        if m == NORM_AT and pending is not None:
                            emit_norm(*pending)
                            pending = None
                    if pending is not None:
                        emit_norm(*pending)
                    pending = (u_ps, hc, hb)
                if pending is not None:
                    emit_norm(*pending)

                # ---- phase E: y = outT.T @ Wproj ----
                for mt in range(8):
                    ps = ppool.tile([128, 1024], f32, tag="s", bufs=2)
                    for half in range(2):
                        for cc in range(4):
                            nc.tensor.matmul(
                                ps[:, 512 * half : 512 * (half + 1)],
                                outt_sb[cc][:, 128 * mt : 128 * (mt + 1)],
                                wproj_sb[cc][:, 512 * half : 512 * (half + 1)],
                                start=(cc == 0), stop=(cc == 3),
                            )
                    y_sb = work.tile([128, 1024], f32, tag="y", bufs=Y_BUFS)
                    nc.scalar.copy(y_sb[:], ps[:])
                    nc.sync.dma_start(y_d[128 * mt : 128 * (mt + 1), :], y_sb[:])

            # right stack unwinds in reverse allocation order
            free_outt()
            free_wproj()
            free_l()
            free_pi32()
            free_kt()
            free_qt()
            free_v()
            free_bpre()
            free_pi()
            free_ones32()

    nc.finalize()
    return nc


def get_nc(repeat=1):
    key = f"nc{repeat}"
    if key not in _cached:
        _cached[key] = _build_nc(repeat)
    return _cached[key]


def kernel(x, Wq, Wk, Wv, Wproj, bproj, Wpre, bpre, pi):
    x = np.asarray(x, np.float32)
    nc = get_nc()
    in_maps = []
    for c in range(NCORES):
        b, hh = c // 2, c % 2
        sl = slice(CH * hh, CH * (hh + 1))
        in_maps.append(
            {
                "xt": np.ascontiguousarray(x[b].T),
                "wpre": np.asarray(Wpre, np.float32),
                "wq": np.ascontiguousarray(np.asarray(Wq, np.float32)[:, sl]) * 0.125,
                "wk": np.ascontiguousarray(np.asarray(Wk, np.float32)[:, sl]),
                "wv": np.ascontiguousarray(np.asarray(Wv, np.float32)[:, sl]),
                "wproj": np.ascontiguousarray(np.asarray(Wproj, np.float32)[sl, :]),
                "bpre": np.asarray(bpre, np.float32),
                "pi": np.asarray(pi, np.float32).reshape(1, 1),
            }
        )
    from concourse.bass_utils import run_bass_kernel_spmd

    res = run_bass_kernel_spmd(nc, in_maps, list(range(NCORES)))
    y = np.empty((B, N, C), np.float32)
    for b in range(B):
        y[b] = (
            res.results[2 * b]["y"]
            + res.results[2 * b + 1]["y"]
            + x[b]
            + np.asarray(bproj, np.float32)[None, :]
        )
    return y

